# revision 1
# baseline (speedup 1.0000x reference)
"""Trainium2 Bass kernel for a 3-layer bidirectional projected-LSTM embedder.

Model (from the reference):
  T=160, B=640, F=40, HID=768, PROJ=256, 3 stacked LSTM-with-projection
  layers per direction (fw, bw).  Per step:
      z = [x_t, h_{t-1}] @ Wk + b            # [B, 4*HID], gate order i,j,f,o
      c = sig(f+1)*c + sig(i)*tanh(j)
      h = (sig(o)*tanh(c)) @ Wp              # [B, PROJ]
  Output = l2norm((concat(fw,bw)[t=0] + concat(fw,bw)[t=T-1]) / 2)  # [B, 512]

Strategy: pure data-parallel over batch (80 per core, 8 cores, no
collectives).  Per core, the three layers run as sequential phases; within a
phase the fw and bw recurrences are interleaved so PE/ACT/DVE overlap.  All
matmuls use float32r (full-rate PE, ~7e-6 elementwise rounding).  Batch-major
z = lhsT.T @ Wk with the activations as the stationary operand and the
(SBUF-resident) weights streaming.  Per-step PE transposes produce the
gate-major h^T needed as next-step stationary operand.  Layer-to-layer h
sequences ping-pong through DRAM.  The final (t0+tT)/2 + l2-normalize is done
on the host in numpy.
"""

import numpy as np

T, B, F = 160, 640, 40
HID, PROJ = 768, 256
NG = 4 * HID          # 3072
NCORES = 8
BC = B // NCORES      # 80
NKH = PROJ // 128     # 2 k-tiles for the recurrent part

_BUILD_CACHE = {}

# Wk column permutation: gate order i,j,f,o -> [i | o | f | j]
_WK_PERM = np.concatenate([np.arange(0, 768), np.arange(2304, 3072), np.arange(1536, 2304), np.arange(768, 1536)])


def _build(use_bias, t_steps, cw=512):
    from contextlib import ExitStack

    import concourse.bass as bass  # noqa: F401
    import concourse.tile as tile
    from concourse import bacc, mybir
    from concourse.masks import make_identity

    f32 = mybir.dt.float32
    f32r = mybir.dt.float32r
    bf16 = mybir.dt.bfloat16
    AF = mybir.ActivationFunctionType

    DIRS = ("fw", "bw")
    CW = cw

    nc = bacc.Bacc(None, target_bir_lowering=False)

    xT = nc.declare_dram_parameter("xT", [F, t_steps * BC], f32r, isOutput=False)
    wk_in = {}
    wp_in = {}
    bb_in = {}
    for d in DIRS:
        for l in range(3):
            ind = F if l == 0 else PROJ
            wk_in[d, l] = nc.declare_dram_parameter(
                f"Wk_{d}{l}", [ind + PROJ, NG], f32r, isOutput=False)
            wp_in[d, l] = nc.declare_dram_parameter(
                f"Wp_{d}{l}", [HID, PROJ], bf16, isOutput=False)
            if use_bias:
                bb_in[d, l] = nc.declare_dram_parameter(
                    f"bb_{d}{l}", [BC, NG], f32, isOutput=False)
    # hT of the top layer at t=0 and t=T-1:  [dir, end, 128, kt, BC]
    out_ends = nc.declare_dram_parameter(
        "out_ends", [2, 2, 128, NKH, BC], f32r, isOutput=True)

    with tile.TileContext(nc) as tc:
        with ExitStack() as top:
            glob = top.enter_context(tc.tile_pool(name="glob", bufs=1))
            dram = top.enter_context(tc.tile_pool(name="dram", bufs=1, space="DRAM"))

            ident = glob.tile([BC, BC], f32)
            make_identity(nc, ident)
            ident_bf = glob.tile([BC, BC], bf16)
            make_identity(nc, ident_bf)

            # layer-to-layer h^T sequences (ping-pong per direction)
            hseq = {}
            for d in DIRS:
                for i in (0, 1):
                    hseq[d, i] = dram.tile([128, NKH, t_steps, BC], f32r,
                                           name=f"hseq_{d}{i}", tag=f"hseq_{d}{i}")

            for l in range(3):
                in_dim = F if l == 0 else PROJ
                with ExitStack() as ph:
                    wpool = ph.enter_context(tc.tile_pool(name=f"w{l}", bufs=1))
                    spool = ph.enter_context(tc.tile_pool(name=f"s{l}", bufs=1))
                    gpool = ph.enter_context(tc.tile_pool(name=f"g{l}", bufs=1))
                    xpool = ph.enter_context(tc.tile_pool(name=f"x{l}", bufs=6))
                    zpool = ph.enter_context(
                        tc.tile_pool(name=f"z{l}", bufs=1, space="PSUM"))
                    apool = ph.enter_context(
                        tc.tile_pool(name=f"a{l}", bufs=1, space="PSUM"))

                    # ---- load weights into SBUF ----
                    # k-tile row spans of Wk: x-part rows then h-part rows
                    if l == 0:
                        kspans = [(0, F), (F, 128), (F + 128, 128)]
                    else:
                        kspans = [(0, 128), (128, 128), (256, 128), (384, 128)]
                    wk_t = {d: [] for d in DIRS}
                    wp_t = {d: [] for d in DIRS}
                    bb_t = {}
                    for d in DIRS:
                        for ki, (r0, rc) in enumerate(kspans):
                            wt = wpool.tile([rc, NG], f32r,
                                            name=f"wk_{d}{l}_{ki}",
                                            tag=f"wk_{d}_{ki}")
                            for c in range(6):
                                nc.sync.dma_start(
                                    out=wt[:, c * 512:(c + 1) * 512],
                                    in_=wk_in[d, l][r0:r0 + rc,
                                                    c * 512:(c + 1) * 512])
                            wk_t[d].append(wt)
                        for ki in range(6):
                            pt = wpool.tile([128, PROJ], bf16,
                                            name=f"wp_{d}{l}_{ki}",
                                            tag=f"wp_{d}_{ki}")
                            nc.sync.dma_start(
                                out=pt, in_=wp_in[d, l][ki * 128:(ki + 1) * 128, :])
                            wp_t[d].append(pt)
                        if use_bias:
                            bt = wpool.tile([BC, NG], f32, name=f"bb_{d}{l}",
                                            tag=f"bb_{d}")
                            for c in range(6):
                                nc.sync.dma_start(
                                    out=bt[:, c * 512:(c + 1) * 512],
                                    in_=bb_in[d, l][:, c * 512:(c + 1) * 512])
                            bb_t[d] = bt

                    # ---- state ----
                    st = {}
                    for d in DIRS:
                        c_sb = spool.tile([BC, HID], f32, name=f"c_{d}{l}",
                                          tag=f"c_{d}")
                        st[d] = [c_sb, None]   # hT produced by step 0

                    for step in range(t_steps):
                        for d in DIRS:
                            t = step if d == "fw" else t_steps - 1 - step
                            c_sb, hT = st[d]

                            if l == 0:
                                xin0 = xpool.tile([F, BC], f32r,
                                                  name=f"xin0_{d}", tag=f"xin_{d}")
                                nc.sync.dma_start(
                                    out=xin0, in_=xT[:, t * BC:(t + 1) * BC])
                                xparts = [xin0]
                            else:
                                xin = xpool.tile([128, NKH * BC], f32r,
                                                 name=f"xin_{d}{l}",
                                                 tag=f"xin_{d}")
                                nc.sync.dma_start(
                                    out=xin.rearrange("p (k b) -> p k b", k=NKH),
                                    in_=hseq[d, (l - 1) % 2][:, :, t, :])
                                xparts = [xin[:, ki * BC:(ki + 1) * BC]
                                          for ki in range(NKH)]
                            if step == 0:
                                lhsts = xparts   # h_{-1} = 0
                            else:
                                lhsts = xparts + [hT[:, ki * BC:(ki + 1) * BC]
                                                  for ki in range(NKH)]

                            # z = [x, h] @ Wk  -> chunks of [BC, CW] in PSUM
                            nch = NG // CW
                            zc = []
                            for c in range(nch):
                                zt = zpool.tile([BC, CW], f32,
                                                name=f"z{c}_{d}{l}", tag=f"z{c}")
                                for ns in range(CW // 512):
                                    cols = slice(c * CW + ns * 512,
                                                 c * CW + (ns + 1) * 512)
                                    for ki, lt in enumerate(lhsts):
                                        nc.tensor.matmul(
                                            zt[:, ns * 512:(ns + 1) * 512],
                                            lt, wk_t[d][ki][:, cols],
                                            start=(ki == 0),
                                            stop=(ki == len(lhsts) - 1))
                                zc.append(zt)

                            # gates (gate g spans z cols [g*HID, (g+1)*HID))
                            gt = {}
                            for g, fn, bias in ((0, AF.Sigmoid, 0.0),
                                                (1, AF.Tanh, 0.0),
                                                (2, AF.Sigmoid, 1.0),
                                                (3, AF.Sigmoid, 0.0)):
                                gt[g] = gpool.tile([BC, HID], f32,
                                                   name=f"g{g}_{d}{l}",
                                                   tag=f"g{g}_{d}")
                                glo, ghi = g * HID, (g + 1) * HID
                                for c in range(glo // CW, (ghi - 1) // CW + 1):
                                    lo, hi = max(glo, c * CW), min(ghi, (c + 1) * CW)
                                    gsrc = zc[c][:, lo - c * CW:hi - c * CW]
                                    if use_bias:
                                        tb = gpool.tile([BC, HID], f32,
                                                        name=f"tb_{d}{l}",
                                                        tag=f"tb_{d}")
                                        nc.vector.tensor_add(
                                            tb[:, 0:hi - lo], gsrc,
                                            bb_t[d][:, lo:hi])
                                        gsrc = tb[:, 0:hi - lo]
                                    nc.scalar.activation(
                                        gt[g][:, lo - glo:hi - glo], gsrc,
                                        fn, bias=bias)

                            # c = sig(f+1)*c + sig(i)*tanh(j)
                            if step == 0:
                                nc.vector.tensor_mul(c_sb, gt[0], gt[1])
                            else:
                                tmp = gpool.tile([BC, HID], f32,
                                                 name=f"tmp_{d}{l}", tag=f"tmp_{d}")
                                nc.vector.tensor_mul(tmp, gt[0], gt[1])
                                nc.vector.tensor_mul(c_sb, gt[2], c_sb)
                                nc.vector.tensor_add(c_sb, c_sb, tmp)
                            tanhc = gpool.tile([BC, HID], f32,
                                               name=f"tanhc_{d}{l}",
                                               tag=f"tanhc_{d}")
                            nc.scalar.activation(tanhc, c_sb, AF.Tanh)
                            s_sb = gpool.tile([BC, HID], bf16,
                                              name=f"s_{d}{l}", tag=f"s_{d}")
                            nc.vector.tensor_mul(s_sb, gt[3], tanhc)

                            # s^T via PE transposes -> [768(6x128), BC]
                            sT_ps = apool.tile([128, 6 * BC], bf16,
                                               name=f"sTp_{d}{l}", tag="sT")
                            for j in range(6):
                                nc.tensor.transpose(
                                    sT_ps[:, j * BC:(j + 1) * BC],
                                    s_sb[:, j * 128:(j + 1) * 128], ident_bf)
                            sT_sb = gpool.tile([128, 6 * BC], bf16,
                                               name=f"sT_{d}{l}", tag=f"sT_{d}")
                            nc.vector.tensor_copy(sT_sb, sT_ps)

                            # h = s @ Wp  [BC, PROJ], then h^T [256(2x128), BC]
                            aux = apool.tile([128, 512], f32,
                                             name=f"aux_{d}{l}", tag="aux")
                            h_ps = aux[0:BC, 0:PROJ]
                            for ki in range(6):
                                nc.tensor.matmul(
                                    h_ps, sT_sb[:, ki * BC:(ki + 1) * BC],
                                    wp_t[d][ki], start=(ki == 0), stop=(ki == 5))
                            h_sb = gpool.tile([BC, PROJ], f32,
                                              name=f"h_{d}{l}", tag=f"h_{d}")
                            nc.vector.tensor_copy(h_sb, h_ps)
                            for j in range(NKH):
                                nc.tensor.transpose(
                                    aux[:, PROJ + j * BC:PROJ + (j + 1) * BC],
                                    h_sb[:, j * 128:(j + 1) * 128], ident)
                            hT_new = spool.tile([128, NKH * BC], f32r,
                                                name=f"hTn_{d}{l}", tag=f"hT_{d}")
                            nc.vector.tensor_copy(hT_new, aux[:, PROJ:PROJ + NKH * BC])
                            st[d][1] = hT_new

                            if l < 2:
                                nc.sync.dma_start(
                                    out=hseq[d, l % 2][:, :, t, :],
                                    in_=hT_new.rearrange("p (k b) -> p k b", k=NKH))
                            else:
                                di = 0 if d == "fw" else 1
                                if t == 0:
                                    nc.sync.dma_start(
                                        out=out_ends[di, 0],
                                        in_=hT_new.rearrange("p (k b) -> p k b",
                                                             k=NKH))
                                if t == t_steps - 1:
                                    nc.sync.dma_start(
                                        out=out_ends[di, 1],
                                        in_=hT_new.rearrange("p (k b) -> p k b",
                                                             k=NKH))

    nc.finalize()
    return nc


def _get_nc(use_bias, t_steps, cw=512):
    key = (use_bias, t_steps, cw)
    if key not in _BUILD_CACHE:
        _BUILD_CACHE[key] = _build(use_bias, t_steps, cw)
    return _BUILD_CACHE[key]


def kernel(**inputs):
    from concourse.bass_utils import run_bass_kernel_spmd

    inp = {k: np.asarray(v, dtype=np.float32) for k, v in inputs.items()}
    batch = inp["batch"]
    assert batch.shape == (T, B, F), batch.shape

    use_bias = any(np.any(inp[f"b_{d}{l}"]) for d in ("fw", "bw") for l in range(3))
    nc = _get_nc(use_bias, T)

    shared = {}
    for d in ("fw", "bw"):
        for l in range(3):
            shared[f"Wk_{d}{l}"] = np.ascontiguousarray(inp[f"Wk_{d}{l}"])
            import ml_dtypes
            shared[f"Wp_{d}{l}"] = np.ascontiguousarray(
                inp[f"Wp_{d}{l}"].astype(ml_dtypes.bfloat16))
            if use_bias:
                shared[f"bb_{d}{l}"] = np.ascontiguousarray(
                    np.broadcast_to(inp[f"b_{d}{l}"], (BC, NG)))

    in_maps = []
    for i in range(NCORES):
        xb = batch[:, i * BC:(i + 1) * BC, :]           # [T, BC, F]
        xT_i = np.ascontiguousarray(
            xb.transpose(2, 0, 1).reshape(F, T * BC))    # [F, T*BC]
        in_maps.append({"xT": xT_i, **shared})

    res = run_bass_kernel_spmd(nc, in_maps, core_ids=list(range(NCORES)))

    # assemble: out_ends [2(dir), 2(end), 128, NKH, BC] -> h [BC, 256]
    h = np.zeros((2, 2, B, PROJ), dtype=np.float32)    # [dir, end, B, PROJ]
    for i in range(NCORES):
        oe = res.results[i]["out_ends"]
        # h[b, kt*128 + p] = oe[.., p, kt, b]
        h[:, :, i * BC:(i + 1) * BC, :] = oe.transpose(0, 1, 4, 3, 2).reshape(
            2, 2, BC, PROJ)

    out0 = np.concatenate([h[0, 0], h[1, 0]], axis=1)   # t = 0
    outT = np.concatenate([h[0, 1], h[1, 1]], axis=1)   # t = T-1
    emb = (out0 + outT) / np.float32(2.0)
    ss = np.maximum(np.sum(emb * emb, axis=-1, keepdims=True), np.float32(1e-12))
    emb = emb / np.sqrt(ss)
    return emb.astype(np.float32)



# revision 14
# speedup vs baseline: 1.4129x; 1.4129x over previous
"""Trainium2 Bass kernel for a 3-layer bidirectional projected-LSTM embedder.

Model (from the reference):
  T=160, B=640, F=40, HID=768, PROJ=256, 3 stacked LSTM-with-projection
  layers per direction (fw, bw).  Per step:
      z = [x_t, h_{t-1}] @ Wk + b            # [B, 4*HID], gate order i,j,f,o
      c = sig(f+1)*c + sig(i)*tanh(j)
      h = (sig(o)*tanh(c)) @ Wp              # [B, PROJ]
  Output = l2norm((concat(fw,bw)[t=0] + concat(fw,bw)[t=T-1]) / 2)  # [B, 512]

Sharding: 4 batch groups x 2 directions = 8 cores, BC=160 per core, one
direction per core.  The bw cores receive the input time-reversed host-side,
so every core runs the SAME program (pure forward scan); the host maps the
two saved end-states back to true time.

Layout: feature-major.  All on-chip tensors are [feature(128-partitions),
batch(160)] tiles; z^T = Wk^T @ [x;h]^T is computed with the bf16 weights as
the 128x128 stationary operand (fast-weight-load keeps LDWEIGHTS hidden
under the 160-column stream) and the f32r activations streaming.  Gates come
out gate-major, so gate activations, the c update, and s = sig(o)*tanh(c)
need no transposes, and h^T = Wp^T @ s^T is again feature-major -- the
recurrence closes with zero per-step transposes.

The three layers run as a pipelined wavefront (super-step n runs layer l at
step n-l), so each layer's ACT/DVE gate chain hides under the other layers'
matmuls.  Wk column order is permuted host-side to [j|i|f|o] so each gate is
one contiguous 2-PSUM-bank block consumed by a single ACT instruction.
Layer-to-layer h stays in SBUF (no DRAM round-trips).  The final
(t0+tT)/2 + l2-normalize is done on the host in numpy.
"""

import numpy as np

T, B, F = 160, 640, 40
HID, PROJ = 768, 256
NG = 4 * HID          # 3072
NCORES = 8
NGRP = 4              # batch groups
BC = B // NGRP        # 160 per core
NT = NG // 128        # 24 gate n-tiles
GPT = 6               # n-tiles per gate
XCH = 16              # x-input DMA chunk (time steps)

_BUILD_CACHE = {}
DEBUG_DUMP = False

# Wk column permutation: gate order i,j,f,o -> [j | i | f | o]
_WK_PERM = np.concatenate([np.arange(768, 1536), np.arange(0, 768),
                           np.arange(1536, 2304), np.arange(2304, 3072)])


def _build(use_bias, t_steps):
    from contextlib import ExitStack

    import concourse.bass as bass  # noqa: F401
    import concourse.tile as tile
    from concourse import bacc, mybir

    f32 = mybir.dt.float32
    f32r = mybir.dt.float32r
    bf16 = mybir.dt.bfloat16
    AF = mybir.ActivationFunctionType

    nc = bacc.Bacc(None, target_bir_lowering=False)

    xT = nc.declare_dram_parameter("xT", [F, t_steps * BC], bf16, isOutput=False)
    wk_in = []
    wp_in = []
    bs_in = []
    for l in range(3):
        ind = F if l == 0 else PROJ
        wk_in.append(nc.declare_dram_parameter(
            f"Wk{l}", [ind + PROJ, NG], bf16, isOutput=False))
        wp_in.append(nc.declare_dram_parameter(
            f"Wp{l}", [HID, PROJ], bf16, isOutput=False))
        if use_bias:
            bs_in.append(nc.declare_dram_parameter(
                f"bs{l}", [128, NT], f32, isOutput=False))
    # h^T of the top layer after the first and last step: [end, 128, m*BC]
    out_ends = nc.declare_dram_parameter(
        "out_ends", [2, 128, 2 * BC], f32r, isOutput=True)
    if DEBUG_DUMP:
        dbg = nc.declare_dram_parameter(
            "dbg", [10, 128, GPT * BC], f32, isOutput=True)

    # k-tile row spans of Wk per layer: x-part rows then h-part rows
    def kspans(l):
        if l == 0:
            return [(0, F)], [(F, 128), (F + 128, 128)]
        return [(0, 128), (128, 128)], [(256, 128), (384, 128)]

    with tile.TileContext(nc) as tc:
        with ExitStack() as top:
            wpool = top.enter_context(tc.tile_pool(name="w", bufs=1))
            st = top.enter_context(tc.tile_pool(name="st", bufs=1))
            hpool = top.enter_context(tc.tile_pool(name="h", bufs=2))
            xpool = top.enter_context(tc.tile_pool(name="x", bufs=2))
            zpool = top.enter_context(
                tc.tile_pool(name="z", bufs=3, space="PSUM"))
            hps = top.enter_context(
                tc.tile_pool(name="hp", bufs=2, space="PSUM"))

            # ---- weights to SBUF ----
            wkx, wkh, wpt, bst = [], [], [], []
            for l in range(3):
                xs, hs = kspans(l)
                xt, ht = [], []
                for ki, (r0, rc) in enumerate(xs + hs):
                    wt = wpool.tile([rc, NG], bf16, name=f"wk{l}_{ki}",
                                    tag=f"wk{l}_{ki}")
                    for c in range(6):
                        nc.sync.dma_start(
                            out=wt[:, c * 512:(c + 1) * 512],
                            in_=wk_in[l][r0:r0 + rc, c * 512:(c + 1) * 512])
                    (xt if ki < len(xs) else ht).append(wt)
                wkx.append(xt)
                wkh.append(ht)
                pt = []
                for ki in range(6):
                    w = wpool.tile([128, PROJ], bf16, name=f"wp{l}_{ki}",
                                   tag=f"wp{l}_{ki}")
                    nc.sync.dma_start(
                        out=w, in_=wp_in[l][ki * 128:(ki + 1) * 128, :])
                    pt.append(w)
                wpt.append(pt)
                if use_bias:
                    bt = wpool.tile([128, NT], f32, name=f"bs{l}", tag=f"bs{l}")
                    nc.sync.dma_start(out=bt, in_=bs_in[l])
                    bst.append(bt)

            # ---- persistent per-layer state ----
            c_sb = [st.tile([128, GPT * BC], f32, name=f"c{l}", tag=f"c{l}")
                    for l in range(3)]
            gsb = [[st.tile([128, GPT * BC], f32, name=f"g{l}_{g}",
                            tag=f"g{l}_{g}") for g in range(4)]
                   for l in range(3)]
            tc_sb = [st.tile([128, GPT * BC], f32, name=f"tc{l}", tag=f"tc{l}")
                     for l in range(3)]
            tmp_sb = [st.tile([128, GPT * BC], f32, name=f"tm{l}", tag=f"tm{l}")
                      for l in range(3)]
            s_sb = [st.tile([128, GPT * BC], bf16, name=f"s{l}", tag=f"s{l}")
                    for l in range(3)]

            h_cur = [None, None, None]   # most recent h^T tile   [128, 2*BC]
            xin = [None, None]           # x chunk double buffer

            def load_xchunk(ch):
                nch = (t_steps + XCH - 1) // XCH
                if ch >= nch:
                    return
                n = min(XCH, t_steps - ch * XCH)
                xt_sb = xpool.tile([F, XCH * BC], bf16, name="xin", tag="xin")
                nc.sync.dma_start(
                    out=xt_sb[:, 0:n * BC],
                    in_=xT[:, ch * XCH * BC:(ch * XCH + n) * BC])
                xin[ch % 2] = xt_sb

            load_xchunk(0)

            def emit_z_act_dve(l, s):
                # gather rhs k-tiles (f32r activations) + matching weights
                if l == 0:
                    if s % XCH == 0:
                        load_xchunk(s // XCH + 1)
                    xt_sb = xin[(s // XCH) % 2]
                    off = (s % XCH) * BC
                    rhs = [xt_sb[:, off:off + BC]]
                else:
                    hb = h_cur[l - 1]
                    rhs = [hb[:, 0:BC], hb[:, BC:2 * BC]]
                wts = list(wkx[l])
                if s > 0:
                    ho = h_cur[l]
                    rhs += [ho[:, 0:BC], ho[:, BC:2 * BC]]
                    wts += wkh[l]
                nk = len(rhs)

                # z matmuls + gate activation, one gate (6 n-tiles) at a time
                for g in range(4):
                    zg = zpool.tile([128, 1024], f32, name=f"z{l}", tag="zg")
                    for n6 in range(GPT):
                        nt = g * GPT + n6
                        dst = zg[:, (n6 // 3) * 512 + (n6 % 3) * BC:]
                        dst = dst[:, 0:BC]
                        for ki in range(nk):
                            # start=True bulk-clears the whole PSUM bank's
                            # has_written bits, racing with the previous
                            # group's drain -- only the first group per bank
                            # may clear; later groups rely on per-element
                            # overwrite-where-unset.
                            nc.tensor.matmul(
                                dst, wts[ki][:, nt * 128:(nt + 1) * 128],
                                rhs[ki],
                                start=(ki == 0 and n6 % 3 == 0),
                                stop=(ki == nk - 1),
                                skip_group_check=(n6 % 3 != 0))
                    fn = AF.Tanh if g == 0 else AF.Sigmoid
                    gd = gsb[l][g]
                    if use_bias:
                        for n6 in range(GPT):
                            nt = g * GPT + n6
                            src = zg[:, (n6 // 3) * 512 + (n6 % 3) * BC:]
                            nc.scalar.activation(
                                gd[:, n6 * BC:(n6 + 1) * BC], src[:, 0:BC],
                                fn, bias=bst[l][:, nt:nt + 1],
                                scale=1.0)
                    else:
                        bias = 1.0 if g == 2 else 0.0
                        src = zg.rearrange("p (b c) -> p b c", b=2)[:, :, 0:3 * BC]
                        dstv = gd.rearrange("p (b c) -> p b c", b=2)
                        nc.scalar.activation(dstv, src, fn, bias=bias)
                    if DEBUG_DUMP and g == 0 and (l, s) in ((0, 0), (0, 1), (1, 0), (1, 1), (2, 0)):
                        slot = {(0, 0): 0, (0, 1): 1, (1, 0): 2,
                                (1, 1): 3, (2, 0): 4}[(l, s)]
                        zt = st.tile([128, GPT * BC], f32, name=f"dbz{slot}",
                                     tag=f"dbz{slot}")
                        nc.vector.tensor_copy(
                            zt.rearrange("p (b c) -> p b c", b=2),
                            zg.rearrange("p (b c) -> p b c", b=2)[:, :, 0:3 * BC])
                        nc.sync.dma_start(out=dbg[slot], in_=zt)

                # c = sig(f+1)*c + sig(i)*tanh(j);  s = sig(o)*tanh(c)
                gj, gi, gf, go = gsb[l]
                if s == 0:
                    nc.vector.tensor_mul(c_sb[l], gi, gj)
                else:
                    nc.vector.tensor_mul(tmp_sb[l], gi, gj)
                    nc.vector.tensor_mul(c_sb[l], gf, c_sb[l])
                    nc.vector.tensor_add(c_sb[l], c_sb[l], tmp_sb[l])
                nc.scalar.activation(tc_sb[l], c_sb[l], AF.Tanh)
                nc.vector.tensor_mul(s_sb[l], go, tc_sb[l])
                if DEBUG_DUMP and (l, s) in ((0, 0), (1, 0)):
                    slot = 8 if l == 0 else 9
                    st9 = st.tile([128, GPT * BC], f32, name=f"dbs{slot}",
                                  tag=f"dbs{slot}")
                    nc.vector.tensor_copy(st9, s_sb[l])
                    nc.sync.dma_start(out=dbg[slot], in_=st9)

            def emit_wp(l, s):
                hp = hps.tile([128, 2 * BC], f32, name="hps", tag="hps")
                for m in range(2):
                    for ki in range(6):
                        nc.tensor.matmul(
                            hp[:, m * BC:(m + 1) * BC],
                            wpt[l][ki][:, m * 128:(m + 1) * 128],
                            s_sb[l][:, ki * BC:(ki + 1) * BC],
                            start=(ki == 0 and m == 0), stop=(ki == 5),
                            skip_group_check=(m == 1))
                hn = hpool.tile([128, 2 * BC], bf16, name=f"hn{l}", tag=f"hn{l}")
                nc.vector.tensor_copy(hn, hp)
                h_cur[l] = hn
                if DEBUG_DUMP and (l, s) in ((0, 0), (1, 0), (0, 1)):
                    slot = {(0, 0): 5, (1, 0): 6, (0, 1): 7}[(l, s)]
                    ht = st.tile([128, 2 * BC], f32, name=f"dbh{slot}",
                                 tag=f"dbh{slot}")
                    nc.vector.tensor_copy(ht, hp)
                    nc.sync.dma_start(out=dbg[slot][:, 0:2 * BC], in_=ht)
                if l == 2 and (s == 0 or s == t_steps - 1):
                    end = 0 if s == 0 else 1
                    oc = st.tile([128, 2 * BC], f32r, name=f"oc{end}",
                                 tag=f"oc{end}")
                    nc.vector.tensor_copy(oc, hp)
                    nc.sync.dma_start(out=out_ends[end], in_=oc)

            # ---- wavefront: super-step n runs layer l at step n-l; each
            # layer's Wp matmul is emitted one z-block later so the PE never
            # waits on the ACT/DVE gate chain.
            wp_queue = []
            for ss in range(t_steps + 2):
                for l in range(3):
                    s = ss - l
                    if not (0 <= s < t_steps):
                        continue
                    # flush any deferred Wp this z-block depends on
                    need = {(l, s - 1), (l - 1, s)}
                    while any(x in wp_queue for x in need):
                        emit_wp(*wp_queue.pop(0))
                    emit_z_act_dve(l, s)
                    wp_queue.append((l, s))
                    if len(wp_queue) > 1:
                        emit_wp(*wp_queue.pop(0))
            while wp_queue:
                emit_wp(*wp_queue.pop(0))

    nc.finalize()
    return nc


def _get_nc(use_bias, t_steps):
    key = (use_bias, t_steps)
    if key not in _BUILD_CACHE:
        _BUILD_CACHE[key] = _build(use_bias, t_steps)
    return _BUILD_CACHE[key]


def make_in_maps(inputs, t_steps=T):
    """Per-core input dicts.  Cores 0-3: fw, batch groups 0-3.
    Cores 4-7: bw (time-reversed input), batch groups 0-3."""
    import ml_dtypes

    inp = {k: np.asarray(v, dtype=np.float32) for k, v in inputs.items()}
    batch = inp["batch"][:t_steps]

    shared = {}
    for d in ("fw", "bw"):
        for l in range(3):
            shared[d, f"Wk{l}"] = np.ascontiguousarray(
                inp[f"Wk_{d}{l}"][:, _WK_PERM].astype(ml_dtypes.bfloat16))
            shared[d, f"Wp{l}"] = np.ascontiguousarray(
                inp[f"Wp_{d}{l}"].astype(ml_dtypes.bfloat16))
            b = inp[f"b_{d}{l}"][_WK_PERM]
            shared[d, f"bs{l}"] = np.ascontiguousarray(
                b.reshape(NT, 128).T.astype(np.float32))

    use_bias = any(np.any(inp[f"b_{d}{l}"])
                   for d in ("fw", "bw") for l in range(3))
    in_maps = []
    for i in range(NCORES):
        d = "fw" if i < NGRP else "bw"
        g = i % NGRP
        xb = batch[:, g * BC:(g + 1) * BC, :]            # [T, BC, F]
        if d == "bw":
            xb = xb[::-1]
        xT_i = np.ascontiguousarray(
            xb.transpose(2, 0, 1).reshape(F, t_steps * BC)
            .astype(ml_dtypes.bfloat16))
        m = {"xT": xT_i}
        for l in range(3):
            m[f"Wk{l}"] = shared[d, f"Wk{l}"]
            m[f"Wp{l}"] = shared[d, f"Wp{l}"]
            if use_bias:
                m[f"bs{l}"] = shared[d, f"bs{l}"]
        in_maps.append(m)
    return in_maps, use_bias


def assemble(results, t_steps=T):
    """results[i]["out_ends"]: [end, m, 128, BC] -> final [B, 2*PROJ] f32."""
    h = np.zeros((2, 2, B, PROJ), dtype=np.float32)   # [dir, end, B, PROJ]
    for i in range(NCORES):
        di, g = (0, i) if i < NGRP else (1, i - NGRP)
        oe = np.asarray(results[i]["out_ends"], dtype=np.float32)
        # oe: [end, p, m*BC];  h[b, m*128 + p] = oe[end, p, m*BC + b]
        h[di, :, g * BC:(g + 1) * BC, :] = oe.reshape(
            2, 128, 2, BC).transpose(0, 3, 2, 1).reshape(2, BC, PROJ)
    # fw end0 = t=0, end1 = t=T-1;  bw (reversed) end0 = t=T-1, end1 = t=0
    out0 = np.concatenate([h[0, 0], h[1, 1]], axis=1)
    outT = np.concatenate([h[0, 1], h[1, 0]], axis=1)
    emb = (out0 + outT) / np.float32(2.0)
    ss = np.maximum(np.sum(emb * emb, axis=-1, keepdims=True),
                    np.float32(1e-12))
    return (emb / np.sqrt(ss)).astype(np.float32)


def kernel(**inputs):
    from concourse.bass_utils import run_bass_kernel_spmd

    batch = np.asarray(inputs["batch"])
    assert batch.shape == (T, B, F), batch.shape
    in_maps, use_bias = make_in_maps(inputs)
    nc = _get_nc(use_bias, T)
    res = run_bass_kernel_spmd(nc, in_maps, core_ids=list(range(NCORES)))
    return assemble(res.results)


# revision 15
# speedup vs baseline: 1.9567x; 1.3849x over previous
"""Trainium2 Bass kernel for a 3-layer bidirectional projected-LSTM embedder.

Model (from the reference):
  T=160, B=640, F=40, HID=768, PROJ=256, 3 stacked LSTM-with-projection
  layers per direction (fw, bw).  Per step:
      z = [x_t, h_{t-1}] @ Wk + b            # [B, 4*HID], gate order i,j,f,o
      c = sig(f+1)*c + sig(i)*tanh(j)
      h = (sig(o)*tanh(c)) @ Wp              # [B, PROJ]
  Output = l2norm((concat(fw,bw)[t=0] + concat(fw,bw)[t=T-1]) / 2)  # [B, 512]

Sharding: 4 batch groups x 2 directions = 8 cores, BC=160 per core, one
direction per core.  The bw cores receive the input time-reversed host-side,
so every core runs the SAME program (pure forward scan); the host maps the
two saved end-states back to true time.

Layout: feature-major.  All on-chip tensors are [feature(128-partitions),
batch(160)] tiles; z^T = Wk^T @ [x;h]^T is computed with the bf16 weights as
the 128x128 stationary operand (fast-weight-load keeps LDWEIGHTS hidden
under the 160-column stream) and the f32r activations streaming.  Gates come
out gate-major, so gate activations, the c update, and s = sig(o)*tanh(c)
need no transposes, and h^T = Wp^T @ s^T is again feature-major -- the
recurrence closes with zero per-step transposes.

The three layers run as a pipelined wavefront (super-step n runs layer l at
step n-l), so each layer's ACT/DVE gate chain hides under the other layers'
matmuls.  Wk column order is permuted host-side to [j|i|f|o] so each gate is
one contiguous 2-PSUM-bank block consumed by a single ACT instruction.
Layer-to-layer h stays in SBUF (no DRAM round-trips).  The final
(t0+tT)/2 + l2-normalize is done on the host in numpy.
"""

import numpy as np

T, B, F = 160, 640, 40
HID, PROJ = 768, 256
NG = 4 * HID          # 3072
NCORES = 8
NGRP = 4              # batch groups
BC = B // NGRP        # 160 per core
NT = NG // 128        # 24 gate n-tiles
GPT = 6               # n-tiles per gate
XCH = 16              # x-input DMA chunk (time steps)

_BUILD_CACHE = {}
DEBUG_DUMP = False

# Wk column permutation: gate order i,j,f,o -> [j | i | f | o]
_WK_PERM = np.concatenate([np.arange(768, 1536), np.arange(0, 768),
                           np.arange(1536, 2304), np.arange(2304, 3072)])


def _build(use_bias, t_steps):
    from contextlib import ExitStack

    import concourse.bass as bass  # noqa: F401
    import concourse.tile as tile
    from concourse import bacc, mybir

    f32 = mybir.dt.float32
    f32r = mybir.dt.float32r
    bf16 = mybir.dt.bfloat16
    AF = mybir.ActivationFunctionType

    nc = bacc.Bacc(None, target_bir_lowering=False)

    xT = nc.declare_dram_parameter("xT", [F, t_steps * BC], bf16, isOutput=False)
    wk_in = []
    wp_in = []
    bs_in = []
    for l in range(3):
        ind = F if l == 0 else PROJ
        wk_in.append(nc.declare_dram_parameter(
            f"Wk{l}", [ind + PROJ, NG], bf16, isOutput=False))
        wp_in.append(nc.declare_dram_parameter(
            f"Wp{l}", [HID, PROJ], bf16, isOutput=False))
        if use_bias:
            bs_in.append(nc.declare_dram_parameter(
                f"bs{l}", [128, NT], f32, isOutput=False))
    # h^T of the top layer after the first and last step: [end, 128, m*BC]
    out_ends = nc.declare_dram_parameter(
        "out_ends", [2, 128, 2 * BC], f32r, isOutput=True)
    if DEBUG_DUMP:
        dbg = nc.declare_dram_parameter(
            "dbg", [10, 128, GPT * BC], f32, isOutput=True)

    # k-tile row spans of Wk per layer: x-part rows then h-part rows
    def kspans(l):
        if l == 0:
            return [(0, F)], [(F, 128), (F + 128, 128)]
        return [(0, 128), (128, 128)], [(256, 128), (384, 128)]

    with tile.TileContext(nc) as tc:
        with ExitStack() as top:
            wpool = top.enter_context(tc.tile_pool(name="w", bufs=1))
            st = top.enter_context(tc.tile_pool(name="st", bufs=1))
            hpool = top.enter_context(tc.tile_pool(name="h", bufs=2))
            xpool = top.enter_context(tc.tile_pool(name="x", bufs=2))
            zpool = top.enter_context(
                tc.tile_pool(name="z", bufs=3, space="PSUM"))
            hps = top.enter_context(
                tc.tile_pool(name="hp", bufs=2, space="PSUM"))

            # ---- weights to SBUF ----
            wkx, wkh, wpt, bst = [], [], [], []
            for l in range(3):
                xs, hs = kspans(l)
                xt, ht = [], []
                for ki, (r0, rc) in enumerate(xs + hs):
                    pad = 128 if rc < 128 else rc
                    wt = wpool.tile([pad, NG], bf16, name=f"wk{l}_{ki}",
                                    tag=f"wk{l}_{ki}")
                    if pad != rc:
                        nc.vector.memset(wt, 0)
                    for c in range(6):
                        nc.sync.dma_start(
                            out=wt[0:rc, c * 512:(c + 1) * 512],
                            in_=wk_in[l][r0:r0 + rc, c * 512:(c + 1) * 512])
                    (xt if ki < len(xs) else ht).append(wt)
                wkx.append(xt)
                wkh.append(ht)
                pt = []
                for ki in range(6):
                    w = wpool.tile([128, PROJ], bf16, name=f"wp{l}_{ki}",
                                   tag=f"wp{l}_{ki}")
                    nc.sync.dma_start(
                        out=w, in_=wp_in[l][ki * 128:(ki + 1) * 128, :])
                    pt.append(w)
                wpt.append(pt)
                if use_bias:
                    bt = wpool.tile([128, NT], f32, name=f"bs{l}", tag=f"bs{l}")
                    nc.sync.dma_start(out=bt, in_=bs_in[l])
                    bst.append(bt)

            # ---- persistent per-layer state ----
            c_sb = [st.tile([128, GPT * BC], f32, name=f"c{l}", tag=f"c{l}")
                    for l in range(3)]
            gsb = [[st.tile([128, GPT * BC], f32, name=f"g{l}_{g}",
                            tag=f"g{l}_{g}") for g in range(4)]
                   for l in range(3)]
            tc_sb = [st.tile([128, GPT * BC], f32, name=f"tc{l}", tag=f"tc{l}")
                     for l in range(3)]
            tmp_sb = [st.tile([128, GPT * BC], f32, name=f"tm{l}", tag=f"tm{l}")
                      for l in range(3)]
            s_sb = [st.tile([128, GPT * BC], bf16, name=f"s{l}", tag=f"s{l}")
                    for l in range(3)]

            h_cur = [None, None, None]   # most recent h^T tile   [128, 2*BC]
            xin = [None, None]           # x chunk double buffer
            xbufs = []                   # pre-zeroed [128, .] chunk buffers

            # x is padded to K=128 with zero rows 40:128 so the layer-0
            # x-part matmul is a normal full-K MM (K=40 stalls the LDW
            # pipeline and trips the HAM throttle every super-step).
            for b in range(2):
                xb = xpool.tile([128, XCH * BC], bf16, name=f"xin{b}",
                                tag=f"xin{b}")
                nc.vector.memset(xb, 0)
                xbufs.append(xb)

            def load_xchunk(ch):
                nch = (t_steps + XCH - 1) // XCH
                if ch >= nch:
                    return
                n = min(XCH, t_steps - ch * XCH)
                xt_sb = xbufs[ch % 2]
                nc.sync.dma_start(
                    out=xt_sb[0:F, 0:n * BC],
                    in_=xT[:, ch * XCH * BC:(ch * XCH + n) * BC])
                xin[ch % 2] = xt_sb

            load_xchunk(0)

            def emit_z_act_dve(l, s):
                # gather rhs k-tiles (f32r activations) + matching weights
                if l == 0:
                    if s % XCH == 0:
                        load_xchunk(s // XCH + 1)
                    xt_sb = xin[(s // XCH) % 2]
                    off = (s % XCH) * BC
                    rhs = [xt_sb[:, off:off + BC]]
                else:
                    hb = h_cur[l - 1]
                    rhs = [hb[:, 0:BC], hb[:, BC:2 * BC]]
                wts = list(wkx[l])
                if s > 0:
                    ho = h_cur[l]
                    rhs += [ho[:, 0:BC], ho[:, BC:2 * BC]]
                    wts += wkh[l]
                nk = len(rhs)

                # z matmuls + gate activation, one gate (6 n-tiles) at a time
                for g in range(4):
                    zg = zpool.tile([128, 1024], f32, name=f"z{l}", tag="zg")
                    for n6 in range(GPT):
                        nt = g * GPT + n6
                        dst = zg[:, (n6 // 3) * 512 + (n6 % 3) * BC:]
                        dst = dst[:, 0:BC]
                        for ki in range(nk):
                            # start=True bulk-clears the whole PSUM bank's
                            # has_written bits, racing with the previous
                            # group's drain -- only the first group per bank
                            # may clear; later groups rely on per-element
                            # overwrite-where-unset.
                            nc.tensor.matmul(
                                dst, wts[ki][:, nt * 128:(nt + 1) * 128],
                                rhs[ki],
                                start=(ki == 0 and n6 % 3 == 0),
                                stop=(ki == nk - 1),
                                skip_group_check=(n6 % 3 != 0))
                    fn = AF.Tanh if g == 0 else AF.Sigmoid
                    gd = gsb[l][g]
                    if use_bias:
                        for n6 in range(GPT):
                            nt = g * GPT + n6
                            src = zg[:, (n6 // 3) * 512 + (n6 % 3) * BC:]
                            nc.scalar.activation(
                                gd[:, n6 * BC:(n6 + 1) * BC], src[:, 0:BC],
                                fn, bias=bst[l][:, nt:nt + 1],
                                scale=1.0)
                    else:
                        bias = 1.0 if g == 2 else 0.0
                        src = zg.rearrange("p (b c) -> p b c", b=2)[:, :, 0:3 * BC]
                        dstv = gd.rearrange("p (b c) -> p b c", b=2)
                        nc.scalar.activation(dstv, src, fn, bias=bias)
                    if DEBUG_DUMP and g == 0 and (l, s) in ((0, 0), (0, 1), (1, 0), (1, 1), (2, 0)):
                        slot = {(0, 0): 0, (0, 1): 1, (1, 0): 2,
                                (1, 1): 3, (2, 0): 4}[(l, s)]
                        zt = st.tile([128, GPT * BC], f32, name=f"dbz{slot}",
                                     tag=f"dbz{slot}")
                        nc.vector.tensor_copy(
                            zt.rearrange("p (b c) -> p b c", b=2),
                            zg.rearrange("p (b c) -> p b c", b=2)[:, :, 0:3 * BC])
                        nc.sync.dma_start(out=dbg[slot], in_=zt)

                # c = sig(f+1)*c + sig(i)*tanh(j);  s = sig(o)*tanh(c)
                gj, gi, gf, go = gsb[l]
                if s == 0:
                    nc.vector.tensor_mul(c_sb[l], gi, gj)
                else:
                    nc.vector.tensor_mul(tmp_sb[l], gi, gj)
                    nc.vector.tensor_mul(c_sb[l], gf, c_sb[l])
                    nc.vector.tensor_add(c_sb[l], c_sb[l], tmp_sb[l])
                nc.scalar.activation(tc_sb[l], c_sb[l], AF.Tanh)
                nc.vector.tensor_mul(s_sb[l], go, tc_sb[l])
                if DEBUG_DUMP and (l, s) in ((0, 0), (1, 0)):
                    slot = 8 if l == 0 else 9
                    st9 = st.tile([128, GPT * BC], f32, name=f"dbs{slot}",
                                  tag=f"dbs{slot}")
                    nc.vector.tensor_copy(st9, s_sb[l])
                    nc.sync.dma_start(out=dbg[slot], in_=st9)

            def emit_wp(l, s):
                hp = hps.tile([128, 2 * BC], f32, name="hps", tag="hps")
                for m in range(2):
                    for ki in range(6):
                        nc.tensor.matmul(
                            hp[:, m * BC:(m + 1) * BC],
                            wpt[l][ki][:, m * 128:(m + 1) * 128],
                            s_sb[l][:, ki * BC:(ki + 1) * BC],
                            start=(ki == 0 and m == 0), stop=(ki == 5),
                            skip_group_check=(m == 1))
                hn = hpool.tile([128, 2 * BC], bf16, name=f"hn{l}", tag=f"hn{l}")
                nc.vector.tensor_copy(hn, hp)
                h_cur[l] = hn
                if DEBUG_DUMP and (l, s) in ((0, 0), (1, 0), (0, 1)):
                    slot = {(0, 0): 5, (1, 0): 6, (0, 1): 7}[(l, s)]
                    ht = st.tile([128, 2 * BC], f32, name=f"dbh{slot}",
                                 tag=f"dbh{slot}")
                    nc.vector.tensor_copy(ht, hp)
                    nc.sync.dma_start(out=dbg[slot][:, 0:2 * BC], in_=ht)
                if l == 2 and (s == 0 or s == t_steps - 1):
                    end = 0 if s == 0 else 1
                    oc = st.tile([128, 2 * BC], f32r, name=f"oc{end}",
                                 tag=f"oc{end}")
                    nc.vector.tensor_copy(oc, hp)
                    nc.sync.dma_start(out=out_ends[end], in_=oc)

            # ---- wavefront: super-step n runs layer l at step n-l; each
            # layer's Wp matmul is emitted one z-block later so the PE never
            # waits on the ACT/DVE gate chain.
            wp_queue = []
            for ss in range(t_steps + 2):
                for l in range(3):
                    s = ss - l
                    if not (0 <= s < t_steps):
                        continue
                    # flush any deferred Wp this z-block depends on
                    need = {(l, s - 1), (l - 1, s)}
                    while any(x in wp_queue for x in need):
                        emit_wp(*wp_queue.pop(0))
                    emit_z_act_dve(l, s)
                    wp_queue.append((l, s))
                    if len(wp_queue) > 1:
                        emit_wp(*wp_queue.pop(0))
            while wp_queue:
                emit_wp(*wp_queue.pop(0))

    nc.finalize()
    return nc


def _get_nc(use_bias, t_steps):
    key = (use_bias, t_steps)
    if key not in _BUILD_CACHE:
        _BUILD_CACHE[key] = _build(use_bias, t_steps)
    return _BUILD_CACHE[key]


def make_in_maps(inputs, t_steps=T):
    """Per-core input dicts.  Cores 0-3: fw, batch groups 0-3.
    Cores 4-7: bw (time-reversed input), batch groups 0-3."""
    import ml_dtypes

    inp = {k: np.asarray(v, dtype=np.float32) for k, v in inputs.items()}
    batch = inp["batch"][:t_steps]

    shared = {}
    for d in ("fw", "bw"):
        for l in range(3):
            shared[d, f"Wk{l}"] = np.ascontiguousarray(
                inp[f"Wk_{d}{l}"][:, _WK_PERM].astype(ml_dtypes.bfloat16))
            shared[d, f"Wp{l}"] = np.ascontiguousarray(
                inp[f"Wp_{d}{l}"].astype(ml_dtypes.bfloat16))
            b = inp[f"b_{d}{l}"][_WK_PERM]
            shared[d, f"bs{l}"] = np.ascontiguousarray(
                b.reshape(NT, 128).T.astype(np.float32))

    use_bias = any(np.any(inp[f"b_{d}{l}"])
                   for d in ("fw", "bw") for l in range(3))
    in_maps = []
    for i in range(NCORES):
        d = "fw" if i < NGRP else "bw"
        g = i % NGRP
        xb = batch[:, g * BC:(g + 1) * BC, :]            # [T, BC, F]
        if d == "bw":
            xb = xb[::-1]
        xT_i = np.ascontiguousarray(
            xb.transpose(2, 0, 1).reshape(F, t_steps * BC)
            .astype(ml_dtypes.bfloat16))
        m = {"xT": xT_i}
        for l in range(3):
            m[f"Wk{l}"] = shared[d, f"Wk{l}"]
            m[f"Wp{l}"] = shared[d, f"Wp{l}"]
            if use_bias:
                m[f"bs{l}"] = shared[d, f"bs{l}"]
        in_maps.append(m)
    return in_maps, use_bias


def assemble(results, t_steps=T):
    """results[i]["out_ends"]: [end, m, 128, BC] -> final [B, 2*PROJ] f32."""
    h = np.zeros((2, 2, B, PROJ), dtype=np.float32)   # [dir, end, B, PROJ]
    for i in range(NCORES):
        di, g = (0, i) if i < NGRP else (1, i - NGRP)
        oe = np.asarray(results[i]["out_ends"], dtype=np.float32)
        # oe: [end, p, m*BC];  h[b, m*128 + p] = oe[end, p, m*BC + b]
        h[di, :, g * BC:(g + 1) * BC, :] = oe.reshape(
            2, 128, 2, BC).transpose(0, 3, 2, 1).reshape(2, BC, PROJ)
    # fw end0 = t=0, end1 = t=T-1;  bw (reversed) end0 = t=T-1, end1 = t=0
    out0 = np.concatenate([h[0, 0], h[1, 1]], axis=1)
    outT = np.concatenate([h[0, 1], h[1, 0]], axis=1)
    emb = (out0 + outT) / np.float32(2.0)
    ss = np.maximum(np.sum(emb * emb, axis=-1, keepdims=True),
                    np.float32(1e-12))
    return (emb / np.sqrt(ss)).astype(np.float32)


def kernel(**inputs):
    from concourse.bass_utils import run_bass_kernel_spmd

    batch = np.asarray(inputs["batch"])
    assert batch.shape == (T, B, F), batch.shape
    in_maps, use_bias = make_in_maps(inputs)
    nc = _get_nc(use_bias, T)
    res = run_bass_kernel_spmd(nc, in_maps, core_ids=list(range(NCORES)))
    return assemble(res.results)
